# revision 1
# baseline (speedup 1.0000x reference)
"""Trainium2 Bass kernel for DecoderRNNTAtt (B=8, T=256, U=64, dims 512, odim 500).

Sharding: data-parallel over batch B across 8 cores (core i handles batch i).
Each core runs: attention-LSTM decoder scan (64 steps) + joint network,
with the joint interleaved into the scan to fill PE idle time.

v2 vs baseline:
  - scan gate matmuls in fp8e4 (e4m3, max 240) with DoubleRow perf mode:
    K=256 per matmul and 0.5 cyc/row -> ~4x less PE stream time.
    Scales: state x16, weights x256, HW x32, softmax-w x128; every psum
    product is x4096, descaled once in the pointwise stt.
  - gates transposed into [128, 4(chunk), 4(gate)] layout via one psum
    quadrant bank (partitions 0/32/64/96) + one [4,512] copy + 4 PE
    transposes, so all LSTM pointwise ops are [128,4]-shaped (fast) and
    z-states come out pre-transposed for the next step's stationary.
  - gate order permuted to (i, f, o, g) so the two tanh scale groups are
    contiguous; embedding contribution + b0 added via DVE (EYB), not PE.
  - joint emitted per-u-pair inside the scan; bias b_out folded in via a
    K=1 matmul so the psum->sbuf copy is a plain cast.
"""

import os
import sys

sys.path.insert(0, "/opt/trn_rl_repo")

from contextlib import ExitStack

import numpy as np
import ml_dtypes

from concourse import bacc, bass, mybir, tile
from concourse.bass_utils import run_bass_kernel_spmd

F32 = mybir.dt.float32
F32R = mybir.dt.float32r
FP8 = mybir.dt.float8e4
E4 = ml_dtypes.float8_e4m3
AF = mybir.ActivationFunctionType
ALU = mybir.AluOpType
AX = mybir.AxisListType
DRow = mybir.MatmulPerfMode.DoubleRow

B, T, U = 8, 256, 64
E = D = A = J = 512
G = 4 * D  # 2048
O = 500
OM = 125
NCORES = 8

SZ = 16.0    # state scale (z doubled, in [-2,2])
SW = 256.0   # weight scale
SHW = 32.0   # HW scale
SSW = 128.0  # softmax-w scale
DESC = 1.0 / (SZ * SW)  # = 1/(SHW*SSW) = 1/4096

# torch gate order (i, f, g, o) -> permuted (i, f, o, g)
PERM = np.r_[0:512, 512:1024, 1536:2048, 1024:1536]

LAST_RESULTS = None
_CACHE = {}


# ----------------------------------------------------------------------------
# host-side packing helpers
# ----------------------------------------------------------------------------

def _pack_k(W):
    """[K, N] -> [128, K//128, N] with [p, c, n] = W[c*128+p, n]."""
    K, N = W.shape
    assert K % 128 == 0
    return np.ascontiguousarray(
        W.reshape(K // 128, 128, N).transpose(1, 0, 2)
    ).astype(np.float32)


def _pack_k8(W, scale):
    """[K, N] -> fp8 [128, K//128, N]."""
    K, N = W.shape
    return np.ascontiguousarray(
        (W.reshape(K // 128, 128, N).transpose(1, 0, 2) * scale)
    ).astype(E4)


def _pack_bias_cols(b, chunk=128):
    n = b.shape[0]
    ncol = (n + chunk - 1) // chunk
    out = np.zeros((128, ncol), np.float32)
    for c in range(ncol):
        seg = b[c * chunk : (c + 1) * chunk]
        out[: seg.shape[0], c] = seg
    return out


def _pack_cg(b):
    """[G] (permuted gate order) -> [128, 4, 4] with [p, c, jb] = b[jb*512+c*128+p]."""
    return np.ascontiguousarray(
        b.reshape(4, 4, 128).transpose(2, 1, 0)
    ).astype(np.float32)


def _prep_inputs(inputs):
    hs = np.asarray(inputs["hs_pad"], np.float32)          # [B, T, E]
    ys = np.asarray(inputs["ys_in_pad"])                   # [B, U] int
    hlens = np.asarray(inputs["hlens"]).astype(np.int64)   # [B]
    emb = np.asarray(inputs["emb"], np.float32)            # [O, E]

    W_ih0 = np.asarray(inputs["W_ih0"], np.float32)        # [G, E + E]
    W_hh0 = np.asarray(inputs["W_hh0"], np.float32)        # [G, D]
    b0 = (np.asarray(inputs["b_ih0"], np.float32)
          + np.asarray(inputs["b_hh0"], np.float32))       # [G]
    W_ih1 = np.asarray(inputs["W_ih1"], np.float32)
    W_hh1 = np.asarray(inputs["W_hh1"], np.float32)
    b1 = (np.asarray(inputs["b_ih1"], np.float32)
          + np.asarray(inputs["b_hh1"], np.float32))

    W_ae = np.asarray(inputs["W_att_enc"], np.float32)     # [E, A]
    b_ae = np.asarray(inputs["b_att_enc"], np.float32)
    W_ad = np.asarray(inputs["W_att_dec"], np.float32)     # [D, A]
    b_ad = np.asarray(inputs["b_att_dec"], np.float32)
    W_le = np.asarray(inputs["W_lin_enc"], np.float32)     # [E, J]
    b_le = np.asarray(inputs["b_lin_enc"], np.float32)
    W_ld = np.asarray(inputs["W_lin_dec"], np.float32)     # [D, J]
    W_out = np.asarray(inputs["W_out"], np.float32)        # [J, O]
    b_out = np.asarray(inputs["b_out"], np.float32)

    # permuted gate blocks
    W_e = W_ih0[PERM, :E]     # [G, E]
    W_a = W_ih0[PERM, E:]     # [G, E]
    W_hh0p = W_hh0[PERM]      # [G, D]
    W_ih1p = W_ih1[PERM]
    W_hh1p = W_hh1[PERM]
    b0p = b0[PERM]
    b1p = b1[PERM]

    shared = {}
    shared["Wae"] = _pack_k(W_ae)                          # [128,4,A]
    shared["bae"] = _pack_bias_cols(b_ae)
    shared["Wle"] = _pack_k(W_le)
    shared["ble"] = _pack_bias_cols(b_le)
    shared["WaT"] = _pack_k(np.ascontiguousarray(W_a.T))   # [128,4,G] f32r
    shared["WeT"] = _pack_k(np.ascontiguousarray(W_e.T))   # [128,4,G]
    shared["b0cg"] = _pack_cg(b0p).reshape(128, 16)        # [p, c*4+jb]
    # fp8 scan weights (hidden state stored doubled -> weights pre-halved)
    shared["Whh08"] = _pack_k8(np.ascontiguousarray(0.5 * W_hh0p.T), SW)
    shared["Wih18"] = _pack_k8(np.ascontiguousarray(0.5 * W_ih1p.T), SW)
    shared["Whh18"] = _pack_k8(np.ascontiguousarray(0.5 * W_hh1p.T), SW)
    shared["Wad8"] = _pack_k8(0.5 * W_ad, SW)              # [128,4,A]
    shared["badT"] = _pack_bias_cols(b_ad)                 # [128,4]
    shared["b1T"] = _pack_cg(b1p)                          # [128,4,4]
    # joint
    shared["Wld"] = _pack_k(0.5 * W_ld)                    # [128,4,J]
    shared["Wout"] = _pack_k(W_out)                        # [128,4,O]
    shared["boutP"] = _pack_bias_cols(b_out, OM)[:OM]      # [125,4]
    shared["id1"] = np.ones((1, 1), np.float32)
    shared["id64"] = np.eye(64, dtype=np.float32)

    in_maps = []
    for b in range(NCORES):
        m = dict(shared)
        hsT = np.ascontiguousarray(hs[b].T)                # [E, T]
        m["hsT"] = _pack_k(hsT)                            # [128,4,T]
        ey = emb[ys[b]]                                    # [U, E] gather
        m["EYT"] = _pack_k(np.ascontiguousarray(ey.T))     # [128,4,U]
        mneg = np.where(np.arange(T) < hlens[b], 0.0, -1e9)
        m["mneg"] = mneg[None, :].astype(np.float32)       # [1,T]
        in_maps.append(m)
    return in_maps


# ----------------------------------------------------------------------------
# kernel builder
# ----------------------------------------------------------------------------

def _build(n_steps=U):
    nc = bacc.Bacc(
        "TRN2", target_bir_lowering=False, debug=False, num_devices=NCORES
    )

    def din(name, shape, dt=F32):
        return nc.dram_tensor(name, list(shape), dt, kind="ExternalInput").ap()

    hsT_d = din("hsT", [128, 4, T], F32R)
    Wae_d = din("Wae", [128, 4, A], F32R)
    bae_d = din("bae", [128, 4])
    Wle_d = din("Wle", [128, 4, J], F32R)
    ble_d = din("ble", [128, 4])
    WaT_d = din("WaT", [128, 4, G], F32R)
    WeT_d = din("WeT", [128, 4, G], F32R)
    b0cg_d = din("b0cg", [128, 16])
    EYT_d = din("EYT", [128, 4, U], F32R)
    Whh08_d = din("Whh08", [128, 4, G], FP8)
    Wih18_d = din("Wih18", [128, 4, G], FP8)
    Whh18_d = din("Whh18", [128, 4, G], FP8)
    Wad8_d = din("Wad8", [128, 4, A], FP8)
    badT_d = din("badT", [128, 4])
    b1T_d = din("b1T", [128, 4, 4])
    Wld_d = din("Wld", [128, 4, J], F32R)
    Wout_d = din("Wout", [128, 4, O], F32R)
    boutP_d = din("boutP", [OM, 4])
    id1_d = din("id1", [1, 1])
    id64_d = din("id64", [64, 64])
    mneg_d = din("mneg", [1, T])

    out_d = nc.dram_tensor(
        "out", [4, OM, n_steps, T], F32, kind="ExternalOutput"
    ).ap()

    with tile.TileContext(nc) as tc, ExitStack() as ctx:
        # ---------------- persistent pool ----------------
        pers = ctx.enter_context(tc.tile_pool(name="pers", bufs=1))
        t_pre = pers.tile([128, 4, T], F32R, name="t_pre", tag="t_pre")
        t_aT = pers.tile([128, 4, T], F32, name="t_aT", tag="t_aT")
        t_HW8 = pers.tile([128, 2, G], FP8, name="t_HW8", tag="t_HW8")
        t_EYB = pers.tile([128, 4, 4, U], F32, name="t_EYB", tag="t_EYB")
        t_Z1 = pers.tile([128, 4, U], F32R, name="t_Z1", tag="t_Z1")
        t_DT = pers.tile([128, 4, U], F32, name="t_DT", tag="t_DT")
        t_Whh08 = pers.tile([128, 4, G], FP8, name="t_Whh08", tag="t_Whh08")
        t_Wih18 = pers.tile([128, 4, G], FP8, name="t_Wih18", tag="t_Wih18")
        t_Whh18 = pers.tile([128, 4, G], FP8, name="t_Whh18", tag="t_Whh18")
        t_Wad8 = pers.tile([128, 4, A], FP8, name="t_Wad8", tag="t_Wad8")
        t_badT = pers.tile([128, 4], F32, name="t_badT", tag="t_badT")
        t_b1T = pers.tile([128, 4, 4], F32, name="t_b1T", tag="t_b1T")
        t_Wld = pers.tile([128, 4, J], F32R, name="t_Wld", tag="t_Wld")
        t_Wout = pers.tile([128, 4, O], F32R, name="t_Wout", tag="t_Wout")
        t_boutP = pers.tile([OM, 4], F32, name="t_boutP", tag="t_boutP")
        t_id1 = pers.tile([1, 1], F32, name="t_id1", tag="t_id1")
        t_id64 = pers.tile([64, 64], F32, name="t_id64", tag="t_id64")
        t_mneg = pers.tile([1, T], F32, name="t_mneg", tag="t_mneg")
        t_c0 = pers.tile([128, 4], F32, name="t_c0", tag="t_c0")
        t_c1 = pers.tile([128, 4], F32, name="t_c1", tag="t_c1")
        t_z8i = pers.tile([128, 4, 16], FP8, name="t_z8i", tag="t_z8i")
        t_z18i = pers.tile([128, 4, 16], FP8, name="t_z18i", tag="t_z18i")

        nc.sync.dma_start(t_Whh08[:], Whh08_d[:])
        nc.sync.dma_start(t_Wih18[:], Wih18_d[:])
        nc.sync.dma_start(t_Whh18[:], Whh18_d[:])
        nc.sync.dma_start(t_Wad8[:], Wad8_d[:])
        nc.sync.dma_start(t_badT[:], badT_d[:])
        nc.sync.dma_start(t_b1T[:], b1T_d[:])
        nc.sync.dma_start(t_Wld[:], Wld_d[:])
        nc.sync.dma_start(t_Wout[:], Wout_d[:])
        nc.sync.dma_start(t_boutP[:], boutP_d[:])
        nc.sync.dma_start(t_id1[:], id1_d[:])
        nc.sync.dma_start(t_id64[:], id64_d[:])
        nc.sync.dma_start(t_mneg[:], mneg_d[:])
        nc.vector.memset(t_c0[:], 0.0)
        nc.vector.memset(t_c1[:], 0.0)
        nc.vector.memset(t_z8i[:], 0.0)
        nc.vector.memset(t_z18i[:], 0.0)

        # psum pool (shared across all phases)
        ps = ctx.enter_context(tc.tile_pool(name="ps", bufs=1, space="PSUM"))
        # scratch pool (per-step small tiles)
        scr = ctx.enter_context(tc.tile_pool(name="scr", bufs=1))

        # ---------------- phase A ----------------
        with tc.tile_pool(name="phA", bufs=1) as pA:
            t_hsT = pA.tile([128, 4, T], F32R, name="t_hsT", tag="t_hsT")
            t_Wae = pA.tile([128, 4, A], F32R, name="t_Wae", tag="t_Wae")
            t_bae = pA.tile([128, 4], F32, name="t_bae", tag="t_bae")
            t_Wle = pA.tile([128, 4, J], F32R, name="t_Wle", tag="t_Wle")
            t_ble = pA.tile([128, 4], F32, name="t_ble", tag="t_ble")
            t_WaT = pA.tile([128, 4, G], F32R, name="t_WaT", tag="t_WaT")
            t_WeT = pA.tile([128, 4, G], F32R, name="t_WeT", tag="t_WeT")
            t_b0cg = pA.tile([128, 16], F32, name="t_b0cg", tag="t_b0cg")
            t_EYT = pA.tile([128, 4, U], F32R, name="t_EYT", tag="t_EYT")

            nc.sync.dma_start(t_hsT[:], hsT_d[:])
            nc.sync.dma_start(t_Wae[:], Wae_d[:])
            nc.sync.dma_start(t_bae[:], bae_d[:])
            nc.sync.dma_start(t_Wle[:], Wle_d[:])
            nc.sync.dma_start(t_ble[:], ble_d[:])
            nc.sync.dma_start(t_WaT[:], WaT_d[:])
            nc.sync.dma_start(t_WeT[:], WeT_d[:])
            nc.sync.dma_start(t_b0cg[:], b0cg_d[:])
            nc.sync.dma_start(t_EYT[:], EYT_d[:])

            # pre_enc[a, t] = tanh(sum_e hs[t,e] Wae[e,a] + bae[a])
            for ca in range(4):
                pe_ps = ps.tile([128, T], F32, name="pe_ps", tag="sm", bufs=1)
                for ce in range(4):
                    nc.tensor.matmul(
                        pe_ps[:],
                        t_Wae[:, ce, ca * 128 : (ca + 1) * 128],
                        t_hsT[:, ce, :],
                        start=(ce == 0),
                        stop=(ce == 3),
                    )
                nc.scalar.activation(
                    t_pre[:, ca, :], pe_ps[:], AF.Tanh,
                    bias=t_bae[:, ca : ca + 1],
                )

            # aT[j, t] = sum_e hs[t,e] Wle[e,j] + ble[j]
            for cj in range(4):
                a_ps = ps.tile([128, T], F32, name="a_ps", tag="sm", bufs=1)
                for ce in range(4):
                    nc.tensor.matmul(
                        a_ps[:],
                        t_Wle[:, ce, cj * 128 : (cj + 1) * 128],
                        t_hsT[:, ce, :],
                        start=(ce == 0),
                        stop=(ce == 3),
                    )
                nc.scalar.activation(
                    t_aT[:, cj, :], a_ps[:], AF.Identity,
                    bias=t_ble[:, cj : cj + 1],
                )

            # HW[t, g] = sum_e hs[t,e] WaT[e,g]  -> fp8 x SHW
            for ct in range(2):
                for jg in range(4):
                    hw_ps = ps.tile([128, 512], F32, name="hw_ps", tag="g",
                                    bufs=1)
                    for ce in range(4):
                        nc.tensor.matmul(
                            hw_ps[:],
                            t_hsT[:, ce, ct * 128 : (ct + 1) * 128],
                            t_WaT[:, ce, jg * 512 : (jg + 1) * 512],
                            start=(ce == 0),
                            stop=(ce == 3),
                        )
                    if jg % 2 == 0:
                        nc.vector.tensor_scalar_mul(
                            t_HW8[:, ct, jg * 512 : (jg + 1) * 512],
                            hw_ps[:], SHW,
                        )
                    else:
                        nc.scalar.activation(
                            t_HW8[:, ct, jg * 512 : (jg + 1) * 512],
                            hw_ps[:], AF.Copy, scale=SHW,
                        )

            # EYB[p, c, jb, u] = (ey[u] @ W_e.T + b0)[jb*512+c*128+p]
            for jg in range(4):
                ey_ps = ps.tile([64, 512], F32, name="ey_ps", tag="sm", bufs=1)
                for ce in range(4):
                    nc.tensor.matmul(
                        ey_ps[:],
                        t_EYT[:, ce, :],
                        t_WeT[:, ce, jg * 512 : (jg + 1) * 512],
                        start=(ce == 0),
                        stop=(ce == 3),
                    )
                eyr = scr.tile([64, 512], F32, name="eyr", tag="eyr", bufs=2)
                nc.vector.tensor_copy(eyr[:], ey_ps[:])
                for c in range(4):
                    eyt = ps.tile([128, 64], F32, name="eyt", tag="tp", bufs=1)
                    nc.tensor.transpose(
                        eyt[:], eyr[0:64, c * 128 : (c + 1) * 128], t_id64[:]
                    )
                    nc.vector.tensor_scalar_add(
                        t_EYB[:, c, jg, :], eyt[:],
                        t_b0cg[:, (c * 4 + jg) : (c * 4 + jg) + 1],
                    )

        # ---------------- phase B: scan with interleaved joint ----------
        z8_prev = t_z8i
        z18_prev = t_z18i
        pairs_done = 0

        jq = []       # pending joint PE units: (pair, closure)
        jq_act = []   # pending joint ACT units: (pair, closure)
        act_done = {}  # pair -> zt pieces emitted

        def drain(n=1):
            for _ in range(n):
                if jq and act_done.get(jq[0][0], 0) >= 4:
                    jq.pop(0)[1]()
                else:
                    break

        def drain_act(n=1):
            for _ in range(n):
                if jq_act:
                    p, fn = jq_act.pop(0)
                    fn()
                    act_done[p] = act_done.get(p, 0) + 1

        def push_joint_pair(p):
            u0 = 2 * p
            zt = scr.tile([128, 4, 2, T], F32R, name="zt", tag="zt", bufs=2)

            def zt_unit(cj):
                def emit():
                    for k in range(2):
                        nc.scalar.activation(
                            zt[:, cj, k, :], t_aT[:, cj, :], AF.Tanh,
                            bias=t_DT[:, cj, u0 + k : u0 + k + 1],
                        )
                return emit
            for cj in range(4):
                jq_act.append((p, zt_unit(cj)))

            def m_unit(m):
                def emit():
                    pj = ps.tile([OM, 2 * T], F32, name="pj", tag="j", bufs=2)
                    for cj in range(4):
                        nc.tensor.matmul(
                            pj[:],
                            t_Wout[:, cj, m * OM : (m + 1) * OM],
                            zt[:, cj, :, :],
                            start=(cj == 0),
                            stop=(cj == 3),
                        )
                    outP = scr.tile([OM, 2, T], F32, name="outP", tag="outP",
                                    bufs=4)
                    if m % 2 == 0:
                        nc.vector.tensor_scalar_add(
                            outP[:], pj[:], t_boutP[:, m : m + 1]
                        )
                    else:
                        nc.scalar.activation(
                            outP[:], pj[:], AF.Identity,
                            bias=t_boutP[:, m : m + 1],
                        )
                    nc.sync.dma_start(
                        out_d[m : m + 1, :, u0 : u0 + 2, :], outP[:]
                    )
                return emit
            for m in range(4):
                jq.append((p, m_unit(m)))

        def emit_dt_chunk(u0):
            # DT[j, u0:u0+8] = (Wld/2) @ Z1[:, u0:u0+8]
            for cj in range(4):
                dps = ps.tile([128, 8], F32, name="dps", tag="tp", bufs=1)
                for cd in range(4):
                    nc.tensor.matmul(
                        dps[:],
                        t_Wld[:, cd, cj * 128 : (cj + 1) * 128],
                        t_Z1[:, cd, u0 : u0 + 8],
                        start=(cd == 0),
                        stop=(cd == 3),
                    )
                nc.vector.tensor_copy(t_DT[:, cj, u0 : u0 + 8], dps[:])

        for u in range(n_steps):
            # ---- attention query q = tanh(z0 @ Wad/2 + b_ad) ----
            qrow = ps.tile([1, A], F32, name="qrow", tag="sm", bufs=1)
            for i in range(2):
                nc.tensor.matmul(
                    qrow[:],
                    z8_prev[:, 2 * i : 2 * i + 2, 0],
                    t_Wad8[:, 2 * i : 2 * i + 2, :],
                    start=(i == 0),
                    stop=(i == 1),
                    perf_mode=DRow,
                )
            q_sb = scr.tile([1, A], F32, name="q_sb", tag="q_sb")
            nc.scalar.activation(q_sb[:], qrow[:], AF.Copy)
            psq_q = ps.tile([128, 4], F32, name="psq_q", tag="tp", bufs=1)
            for c in range(4):
                nc.tensor.transpose(
                    psq_q[:, c : c + 1],
                    q_sb[0:1, c * 128 : (c + 1) * 128],
                    t_id1[:],
                )
            qpre = scr.tile([128, 4], F32, name="qpre", tag="qpre")
            nc.vector.scalar_tensor_tensor(
                qpre[:], psq_q[:], DESC, t_badT[:], ALU.mult, ALU.add
            )
            qs = scr.tile([128, 4], F32, name="qs", tag="qs")
            nc.scalar.activation(qs[:], qpre[:], AF.Tanh)
            qT = scr.tile([128, 4], F32R, name="qT", tag="qT", bufs=2)
            nc.vector.tensor_copy(qT[:], qs[:])

            # ---- e[t] = sum_a pre_enc[a,t] q[a] ----
            eps = ps.tile([1, T], F32, name="eps", tag="sm", bufs=1)
            for ca in range(4):
                nc.tensor.matmul(
                    eps[:],
                    qT[:, ca : ca + 1],
                    t_pre[:, ca, :],
                    start=(ca == 0),
                    stop=(ca == 3),
                )

            # ---- gates0 Whh0 part (PE works during softmax latency) ----
            g0 = ps.tile([1, G], F32, name="g0", tag="g", bufs=1)
            for jb in range(4):
                sl = slice(jb * 512, (jb + 1) * 512)
                for i in range(2):
                    nc.tensor.matmul(
                        g0[0:1, sl],
                        z8_prev[:, 2 * i : 2 * i + 2, 0],
                        t_Whh08[:, 2 * i : 2 * i + 2, sl],
                        start=(i == 0),
                        stop=False,
                        perf_mode=DRow,
                    )

            drain(1)

            # ---- softmax (DVE/ACT) ----
            em = scr.tile([1, T], F32, name="em", tag="em")
            nc.vector.tensor_tensor(em[:], eps[:], t_mneg[:], ALU.add)
            ngmx = scr.tile([1, 1], F32, name="ngmx", tag="ngmx")
            nc.vector.tensor_reduce(ngmx[:], em[:], AX.X, ALU.max, negate=True)
            wsc = scr.tile([1, T], F32, name="wsc", tag="wsc")
            sume = scr.tile([1, 1], F32, name="sume", tag="sume")
            nc.scalar.activation(
                wsc[:], em[:], AF.Exp, bias=ngmx[:], accum_out=sume[:]
            )
            rinv = scr.tile([1, 1], F32, name="rinv", tag="rinv")
            nc.vector.reciprocal(rinv[:], sume[:])
            rinvS = scr.tile([1, 1], F32, name="rinvS", tag="rinvS")
            nc.vector.tensor_scalar_mul(rinvS[:], rinv[:], SSW)
            wrow = scr.tile([1, T], F32, name="wrow", tag="wrow")
            nc.vector.tensor_scalar_mul(wrow[:], wsc[:], rinvS[:])
            wps = ps.tile([128, 2], F32, name="wps", tag="tp", bufs=1)
            for ct in range(2):
                nc.tensor.transpose(
                    wps[:, ct : ct + 1],
                    wrow[0:1, ct * 128 : (ct + 1) * 128],
                    t_id1[:],
                )
            w8 = scr.tile([128, 2, 16], FP8, name="w8", tag="w8", bufs=2)
            nc.vector.tensor_copy(w8[:, :, 0], wps[:])

            # ---- gates0 w@HW part ----
            for jb in range(4):
                sl = slice(jb * 512, (jb + 1) * 512)
                nc.tensor.matmul(
                    g0[0:1, sl],
                    w8[:, :, 0],
                    t_HW8[:, :, sl],
                    start=False,
                    stop=True,
                    perf_mode=DRow,
                )

            drain(1)

            # ---- cell 0 pointwise (transposed layout) ----
            g0sb = scr.tile([1, G], F32, name="g0sb", tag="gsb", bufs=2)
            nc.scalar.activation(g0sb[0:1, 0:1024], g0[0:1, 0:1024], AF.Copy)
            nc.vector.tensor_copy(g0sb[0:1, 1024:2048], g0[0:1, 1024:2048])
            psq0 = ps.tile([128, 4, 4], F32, name="psq0", tag="tp", bufs=1)
            for jb in range(4):
                for c in range(4):
                    nc.tensor.transpose(
                        psq0[:, c, jb : jb + 1],
                        g0sb[0:1, jb * 512 + c * 128 : jb * 512 + (c + 1) * 128],
                        t_id1[:],
                    )
            tmp0 = scr.tile([128, 4, 4], F32, name="tmp0", tag="tmp0")
            nc.vector.scalar_tensor_tensor(
                tmp0[:], psq0[:], DESC, t_EYB[:, :, :, u], ALU.mult, ALU.add
            )
            th0 = scr.tile([128, 4, 4], F32, name="th0", tag="th0")
            nc.scalar.activation(th0[:, :, 0:3], tmp0[:, :, 0:3], AF.Tanh,
                                 scale=0.5)
            nc.scalar.activation(th0[:, :, 3:4], tmp0[:, :, 3:4], AF.Tanh)
            tt1 = scr.tile([128, 4], F32, name="tt1", tag="tt1")
            nc.vector.scalar_tensor_tensor(
                tt1[:], th0[:, :, 1], 1.0, t_c0[:], ALU.add, ALU.mult
            )
            tt2 = scr.tile([128, 4], F32, name="tt2", tag="tt2")
            nc.vector.scalar_tensor_tensor(
                tt2[:], th0[:, :, 0], 1.0, th0[:, :, 3], ALU.add, ALU.mult
            )
            nc.vector.scalar_tensor_tensor(
                t_c0[:], tt1[:], 0.5, tt2[:], ALU.mult, ALU.add
            )
            thc0 = scr.tile([128, 4], F32, name="thc0", tag="thc0")
            nc.scalar.activation(thc0[:], t_c0[:], AF.Tanh, scale=0.5)
            z0f = scr.tile([128, 4], F32, name="z0f", tag="z0f")
            nc.vector.scalar_tensor_tensor(
                z0f[:], th0[:, :, 2], 1.0, thc0[:], ALU.add, ALU.mult
            )
            z08 = scr.tile([128, 4, 16], FP8, name="z08", tag="z08", bufs=2)
            nc.vector.tensor_scalar_mul(z08[:, :, 0], z0f[:], SZ)
            drain_act(2)

            # ---- gates1 (bank reuse after cell0 copies) ----
            g1 = ps.tile([1, G], F32, name="g1", tag="g", bufs=1)
            for jb in range(4):
                sl = slice(jb * 512, (jb + 1) * 512)
                for i in range(2):
                    nc.tensor.matmul(
                        g1[0:1, sl],
                        z18_prev[:, 2 * i : 2 * i + 2, 0],
                        t_Whh18[:, 2 * i : 2 * i + 2, sl],
                        start=(i == 0),
                        stop=False,
                        perf_mode=DRow,
                    )
                for i in range(2):
                    nc.tensor.matmul(
                        g1[0:1, sl],
                        z08[:, 2 * i : 2 * i + 2, 0],
                        t_Wih18[:, 2 * i : 2 * i + 2, sl],
                        start=False,
                        stop=(i == 1),
                        perf_mode=DRow,
                    )

            drain(1)

            # ---- cell 1 pointwise ----
            g1sb = scr.tile([1, G], F32, name="g1sb", tag="gsb", bufs=2)
            nc.scalar.activation(g1sb[0:1, 0:1024], g1[0:1, 0:1024], AF.Copy)
            nc.vector.tensor_copy(g1sb[0:1, 1024:2048], g1[0:1, 1024:2048])
            psq1 = ps.tile([128, 4, 4], F32, name="psq1", tag="tp", bufs=1)
            for jb in range(4):
                for c in range(4):
                    nc.tensor.transpose(
                        psq1[:, c, jb : jb + 1],
                        g1sb[0:1, jb * 512 + c * 128 : jb * 512 + (c + 1) * 128],
                        t_id1[:],
                    )
            tmp1 = scr.tile([128, 4, 4], F32, name="tmp1", tag="tmp1")
            nc.vector.scalar_tensor_tensor(
                tmp1[:], psq1[:], DESC, t_b1T[:], ALU.mult, ALU.add
            )
            th1 = scr.tile([128, 4, 4], F32, name="th1", tag="th1")
            nc.scalar.activation(th1[:, :, 0:3], tmp1[:, :, 0:3], AF.Tanh,
                                 scale=0.5)
            nc.scalar.activation(th1[:, :, 3:4], tmp1[:, :, 3:4], AF.Tanh)
            tt3 = scr.tile([128, 4], F32, name="tt3", tag="tt1")
            nc.vector.scalar_tensor_tensor(
                tt3[:], th1[:, :, 1], 1.0, t_c1[:], ALU.add, ALU.mult
            )
            tt4 = scr.tile([128, 4], F32, name="tt4", tag="tt2")
            nc.vector.scalar_tensor_tensor(
                tt4[:], th1[:, :, 0], 1.0, th1[:, :, 3], ALU.add, ALU.mult
            )
            nc.vector.scalar_tensor_tensor(
                t_c1[:], tt3[:], 0.5, tt4[:], ALU.mult, ALU.add
            )
            thc1 = scr.tile([128, 4], F32, name="thc1", tag="thc0")
            nc.scalar.activation(thc1[:], t_c1[:], AF.Tanh, scale=0.5)
            z1f = scr.tile([128, 4], F32, name="z1f", tag="z1f")
            nc.vector.scalar_tensor_tensor(
                z1f[:], th1[:, :, 2], 1.0, thc1[:], ALU.add, ALU.mult
            )
            z18 = scr.tile([128, 4, 16], FP8, name="z18", tag="z18", bufs=2)
            nc.vector.tensor_scalar_mul(z18[:, :, 0], z1f[:], SZ)
            nc.vector.tensor_copy(t_Z1[:, :, u], z1f[:])

            z8_prev = z08
            z18_prev = z18
            drain_act(1)

            # ---- interleaved joint: refill queue ----
            if u % 8 == 7:
                emit_dt_chunk(u - 7)
                for p in range(u // 2 - 3, u // 2 + 1):
                    push_joint_pair(p)
                    pairs_done += 1

        # tail: remaining joint units
        drain_act(len(jq_act))
        drain(len(jq))

    nc.compile()
    return nc


# ----------------------------------------------------------------------------
# entry point
# ----------------------------------------------------------------------------

def kernel(**inputs):
    global LAST_RESULTS
    if "nc" not in _CACHE:
        _CACHE["nc"] = _build(U)
    nc = _CACHE["nc"]
    in_maps = _prep_inputs(inputs)
    res = run_bass_kernel_spmd(
        nc, in_maps, list(range(NCORES)),
        trace=bool(int(os.environ.get("KBENCH_TRACE", "0"))),
    )
    LAST_RESULTS = res
    outs = []
    for c in range(NCORES):
        o = res.results[c]["out"]              # [4, 125, U, T]
        o = o.reshape(O, U, T).transpose(2, 1, 0)  # [T, U, O]
        outs.append(np.ascontiguousarray(o))
    full = np.stack(outs, axis=0).astype(np.float32)  # [B, T, U, O]
    return full

